# revision 27
# baseline (speedup 1.0000x reference)
"""AFT-full kernel for one TRN2 chip (8 NeuronCores), data-parallel over batch.

Math (per batch b):
    q = x @ Wq.T + bq ; k = x @ Wk.T + bk ; v = x @ Wv.T + bv
    ek = exp(k); eb = exp(pos_bias)
    out = sigmoid(q) * (eb @ (ek*v)) / (eb @ ek)

Sharding: batch 64 -> 8 cores x 8 batches. Weights + pos_bias replicated.
Host-side layout prep: x is fed per-batch transposed (xT[b] = x[b].T) so the
d-contraction matmuls have d on partitions; pos_bias is fed transposed so it
can be used directly as the stationary operand of the j-contraction.
"""

import numpy as np

D = 512          # d_model
N = 1024         # sequence length
BS = 64          # global batch
NCORES = 8
BPC = BS // NCORES   # batches per core
P = 128          # partitions
DC = D // P      # 4 chunks of d
NT = N // P      # 8 tiles of n

# matmul operand dtype mode: "f32r" (relaxed fp32, full PE rate at N>=256),
# "f32" (4x slower, exact), "bf16" handled by separate build path if needed.
MM_MODE = "bf16"

_CACHE = {}


def _build(with_bias: bool):
    from contextlib import ExitStack

    import concourse.bass as bass
    import concourse.tile as tile
    from concourse import bacc, mybir

    f32 = mybir.dt.float32
    # matmul-operand dtype: tiles feeding the PE are typed fmm so the BIR
    # verifier sees properly-rounded producers; fmm==float32r runs the PE at
    # full rate for N>=256 moving operands.
    fmm = {"f32r": mybir.dt.float32r,
           "bf16": mybir.dt.bfloat16,
           "f32": f32}[MM_MODE]
    AF = mybir.ActivationFunctionType

    def mm_ap(ap):
        return ap

    nc = bacc.Bacc("TRN2", target_bir_lowering=False, debug=False,
                   num_devices=NCORES)

    # x and W arrive pre-permuted from the host as [.., P, DC, cols] so every
    # DMA lands contiguously per partition (full HBM bandwidth):
    #   dev[p, c, col] = T[c*P + p, col]
    xT = nc.declare_dram_parameter("xT", [BPC, P, DC, N], fmm, isOutput=False)
    wqT = nc.declare_dram_parameter("wqT", [P, DC, D], fmm, isOutput=False)
    wkT = nc.declare_dram_parameter("wkT", [P, DC, D], fmm, isOutput=False)
    wvT = nc.declare_dram_parameter("wvT", [P, DC, D], fmm, isOutput=False)
    pbT = nc.declare_dram_parameter("pbT", [N, N], f32, isOutput=False)
    if with_bias:
        bias = nc.declare_dram_parameter("bias", [3, D], fmm, isOutput=False)
    out = nc.declare_dram_parameter("out", [BPC, N, D], f32, isOutput=True)

    with tile.TileContext(nc) as tc, ExitStack() as ctx:
        wpool = ctx.enter_context(tc.tile_pool(name="w", bufs=1))
        ebpool = ctx.enter_context(tc.tile_pool(name="eb", bufs=1))
        stg = ctx.enter_context(tc.tile_pool(name="stg", bufs=2))
        xpool = ctx.enter_context(tc.tile_pool(name="x", bufs=2))
        ekpool = ctx.enter_context(tc.tile_pool(name="ek", bufs=2))
        ekvpool = ctx.enter_context(tc.tile_pool(name="ekv", bufs=2))
        spool = ctx.enter_context(tc.tile_pool(name="small", bufs=2))
        opool = ctx.enter_context(tc.tile_pool(name="out", bufs=3))
        ps1 = ctx.enter_context(
            tc.tile_pool(name="ps1", bufs=4, space=bass.MemorySpace.PSUM))
        ps2 = ctx.enter_context(
            tc.tile_pool(name="ps2", bufs=4, space=bass.MemorySpace.PSUM))

        # ---- replicated constants -------------------------------------
        # weights stored [p, chunk, e]: partition = d within chunk.
        # Chunked DMAs so the first matmul only waits on ~512KB, not 7MB.
        # issue the startup DMAs from different engines so the ~600ns
        # issue instructions don't serialize on one queue
        wq_t = wpool.tile([P, DC, D], fmm, tag="wq")
        wk_t = wpool.tile([P, DC, D], fmm, tag="wk")
        wv_t = wpool.tile([P, DC, D], fmm, tag="wv")
        nc.sync.dma_start(wk_t[:], wkT.ap())

        if with_bias:
            b_t = wpool.tile([1, 3, D], fmm, tag="bias")
            nc.sync.dma_start(b_t[:], bias.ap().rearrange("t e -> 1 t e"))
            ones_t = wpool.tile([1, P], fmm, tag="ones")
            nc.gpsimd.memset(ones_t[:], 1.0)

        eb_t = ebpool.tile([P, NT, N], fmm, tag="ebt")

        # ---- per-batch pipeline ---------------------------------------
        for b in range(BPC):
            xt = xpool.tile([P, DC, N], fmm, tag="xt")
            if b == 0:
                # first batch: spread the ~3MB startup set (wk + x chunks +
                # wv) over all three DMA issue queues in consumption order;
                # per-queue bandwidth is only ~145GB/s
                xv = xT.ap()[b]
                nc.scalar.dma_start(xt[:, 0, :], xv[:, 0, :])
                nc.gpsimd.dma_start(xt[:, 1, :], xv[:, 1, :])
                nc.scalar.dma_start(xt[:, 2, :], xv[:, 2, :])
                nc.sync.dma_start(xt[:, 3, :], xv[:, 3, :])
                nc.gpsimd.dma_start(wv_t[:], wvT.ap())
            else:
                nc.sync.dma_start(xt[:], xT.ap()[b])

            ek = ekpool.tile([P, NT, D], fmm, tag="ek")
            ekv = ekvpool.tile([P, NT, D], fmm, tag="ekv")

            # stage 1: k, v projections; ek = exp(k); ekv = ek * v
            for t in range(NT):
                kps = ps1.tile([P, D], f32, tag="ps1")
                for dc in range(DC):
                    nc.tensor.matmul(
                        kps[:], mm_ap(xt[:, dc, t * P:(t + 1) * P]),
                        mm_ap(wk_t[:, dc, :]),
                        start=(dc == 0), stop=(dc == DC - 1 and not with_bias))
                if with_bias:
                    nc.tensor.matmul(
                        kps[:], mm_ap(ones_t[0:1, :]), mm_ap(b_t[0:1, 1, :]),
                        start=False, stop=True)
                vps = ps1.tile([P, D], f32, tag="ps1")
                for dc in range(DC):
                    nc.tensor.matmul(
                        vps[:], mm_ap(xt[:, dc, t * P:(t + 1) * P]),
                        mm_ap(wv_t[:, dc, :]),
                        start=(dc == 0), stop=(dc == DC - 1 and not with_bias))
                if with_bias:
                    nc.tensor.matmul(
                        vps[:], mm_ap(ones_t[0:1, :]), mm_ap(b_t[0:1, 2, :]),
                        start=False, stop=True)
                nc.scalar.activation(ek[:, t, :], kps[:], AF.Exp)
                nc.vector.tensor_mul(ekv[:, t, :], vps[:], ek[:, t, :])

                if b == 0 and t == 1:
                    # deferred constants: needed from stage 2 onwards;
                    # emitting them here keeps the startup DMA critical path
                    # minimal while still landing before stage 2 needs them.
                    # pos_bias rides the gpsimd SWDGE queue.
                    nc.sync.dma_start(wq_t[:], wqT.ap())
                    for jc in range(NT):
                        pb_stage = stg.tile([P, N], f32, tag="pbstg")
                        nc.gpsimd.dma_start(
                            pb_stage[:], pbT.ap()[jc * P:(jc + 1) * P, :])
                        nc.scalar.activation(
                            eb_t[:, jc, :], pb_stage[:], AF.Exp)

            # stage 2: q first (so sigmoid overlaps den/num matmuls),
            # then den = eb@ek and num = eb@ekv; combine and store
            for t in range(NT):
                qps = ps1.tile([P, D], f32, tag="ps1")
                for dc in range(DC):
                    nc.tensor.matmul(
                        qps[:], mm_ap(xt[:, dc, t * P:(t + 1) * P]),
                        mm_ap(wq_t[:, dc, :]),
                        start=(dc == 0), stop=(dc == DC - 1 and not with_bias))
                if with_bias:
                    nc.tensor.matmul(
                        qps[:], mm_ap(ones_t[0:1, :]), mm_ap(b_t[0:1, 0, :]),
                        start=False, stop=True)
                sig = spool.tile([P, D], f32, tag="sig")
                nc.scalar.activation(sig[:], qps[:], AF.Sigmoid)
                # den/num interleaved per j-chunk: adjacent matmuls share the
                # same stationary ebT tile, so walrus ldw-opt can drop every
                # second LDWEIGHTS
                dps = ps2.tile([P, D], f32, tag="ps2")
                nps = ps2.tile([P, D], f32, tag="ps2")
                for jc in range(NT):
                    nc.tensor.matmul(
                        dps[:], mm_ap(eb_t[:, jc, t * P:(t + 1) * P]),
                        mm_ap(ek[:, jc, :]),
                        start=(jc == 0), stop=(jc == NT - 1))
                    nc.tensor.matmul(
                        nps[:], mm_ap(eb_t[:, jc, t * P:(t + 1) * P]),
                        mm_ap(ekv[:, jc, :]),
                        start=(jc == 0), stop=(jc == NT - 1))
                rec = spool.tile([P, D], f32, tag="rec")
                nc.vector.reciprocal_approx_fast(rec[:], dps[:])
                ot = opool.tile([P, D], f32, tag="ot")
                nc.vector.tensor_mul(ot[:], nps[:], rec[:])
                nc.vector.tensor_mul(ot[:], ot[:], sig[:])
                nc.sync.dma_start(out.ap()[b, t * P:(t + 1) * P, :], ot[:])

    nc.compile()
    return nc


_LDW_OPT = False


def _patch_ldw_opt():
    """Flip walrus's --enable-ldw-opt so duplicate LDWEIGHTS of the same
    stationary tile (den/num pairs in stage 2) collapse to one load."""
    import concourse.bass_utils as bu
    if getattr(bu, "_aft_ldw_patched", False):
        return
    orig = bu.run_command

    def patched(cmd, *a, **kw):
        if isinstance(cmd, list):
            cmd = [c.replace("--enable-ldw-opt=false", "--enable-ldw-opt=true")
                   if isinstance(c, str) else c for c in cmd]
        return orig(cmd, *a, **kw)

    bu.run_command = patched
    bu._aft_ldw_patched = True


def _run(inputs, trace=False, **spmd_kwargs):
    from concourse.bass_utils import run_bass_kernel_spmd
    if _LDW_OPT:
        _patch_ldw_opt()

    x = np.ascontiguousarray(np.asarray(inputs["x"], dtype=np.float32))
    Wq = np.asarray(inputs["Wq"], dtype=np.float32)
    Wk = np.asarray(inputs["Wk"], dtype=np.float32)
    Wv = np.asarray(inputs["Wv"], dtype=np.float32)
    bq = np.asarray(inputs["bq"], dtype=np.float32)
    bk = np.asarray(inputs["bk"], dtype=np.float32)
    bv = np.asarray(inputs["bv"], dtype=np.float32)
    pb = np.asarray(inputs["pos_bias"], dtype=np.float32)

    if MM_MODE == "bf16":
        import ml_dtypes
        _mt = ml_dtypes.bfloat16
    else:
        _mt = np.float32

    def _perm(wT):
        # [D, cols] -> [P, DC, cols] with dev[p, c, :] = wT[c*P + p, :]
        cols = wT.shape[1]
        return np.ascontiguousarray(
            wT.reshape(DC, P, cols).transpose(1, 0, 2)).astype(_mt)

    # x[b].T pre-permuted: xT[b, p, c, n] = x[b].T[c*P + p, n]
    xT = np.ascontiguousarray(
        x.transpose(0, 2, 1).reshape(BS, DC, P, N).transpose(0, 2, 1, 3)
    ).astype(_mt)                                                # [BS, P, DC, N]
    wqT = _perm(Wq.T)                                            # [P, DC, D]
    wkT = _perm(Wk.T)
    wvT = _perm(Wv.T)
    pbT = np.ascontiguousarray(pb.T)                             # [j, i]

    with_bias = bool(np.any(bq) or np.any(bk) or np.any(bv))
    key = ("nc", with_bias, MM_MODE)
    if key not in _CACHE:
        _CACHE[key] = _build(with_bias)
    nc = _CACHE[key]

    in_maps = []
    for c in range(NCORES):
        m = {
            "xT": xT[c * BPC:(c + 1) * BPC],
            "wqT": wqT, "wkT": wkT, "wvT": wvT,
            "pbT": pbT,
        }
        if with_bias:
            m["bias"] = np.ascontiguousarray(np.stack([bq, bk, bv])).astype(_mt)
        in_maps.append(m)

    res = run_bass_kernel_spmd(nc, in_maps, core_ids=list(range(NCORES)),
                               trace=trace, **spmd_kwargs)
    out = np.concatenate([r["out"] for r in res.results], axis=0)
    return out.astype(np.float32, copy=False), res


def kernel(**inputs) -> np.ndarray:
    out, _ = _run(inputs, trace=False)
    return out


# revision 37
# speedup vs baseline: 1.3127x; 1.3127x over previous
"""AFT-full kernel for one TRN2 chip (8 NeuronCores), data-parallel over batch.

Math (per batch b):
    q = x @ Wq.T + bq ; k = x @ Wk.T + bk ; v = x @ Wv.T + bv
    ek = exp(k); eb = exp(pos_bias)
    out = sigmoid(q) * (eb @ (ek*v)) / (eb @ ek)

Sharding: batch 64 -> 8 cores x 8 batches. Weights + pos_bias replicated.
Host-side layout prep: x is fed per-batch transposed (xT[b] = x[b].T) so the
d-contraction matmuls have d on partitions; pos_bias is fed transposed so it
can be used directly as the stationary operand of the j-contraction.
"""

import numpy as np

D = 512          # d_model
N = 1024         # sequence length
BS = 64          # global batch
NCORES = 8
BPC = BS // NCORES   # batches per core
P = 128          # partitions
DC = D // P      # 4 chunks of d
NT = N // P      # 8 tiles of n

# matmul operand dtype mode: "f32r" (relaxed fp32, full PE rate at N>=256),
# "f32" (4x slower, exact), "bf16" handled by separate build path if needed.
MM_MODE = "bf16"

# k/q projections in fp8e4m3 + DoubleRow (K=256 per pass). Their quantization
# error is damped by exp/sigmoid (k,q ~ +-0.1 so |d ek| ~ |dk|*ek ~ 1e-3);
# v/den/num stay bf16. Weights are pre-scaled by FP8_SCALE on the host and
# un-scaled for free via the activation `scale` argument.
FP8_PROJ = True
FP8_SCALE = 128.0

_CACHE = {}


def _build(with_bias: bool, fp8: bool):
    from contextlib import ExitStack

    import concourse.bass as bass
    import concourse.tile as tile
    from concourse import bacc, mybir

    f32 = mybir.dt.float32
    # matmul-operand dtype: tiles feeding the PE are typed fmm so the BIR
    # verifier sees properly-rounded producers; fmm==float32r runs the PE at
    # full rate for N>=256 moving operands.
    fmm = {"f32r": mybir.dt.float32r,
           "bf16": mybir.dt.bfloat16,
           "f32": f32}[MM_MODE]
    AF = mybir.ActivationFunctionType

    def mm_ap(ap):
        return ap

    nc = bacc.Bacc("TRN2", target_bir_lowering=False, debug=False,
                   num_devices=NCORES)

    # x and W arrive pre-permuted from the host as [.., P, DC, cols] so every
    # DMA lands contiguously per partition (full HBM bandwidth):
    #   dev[p, c, col] = T[c*P + p, col]
    f8 = mybir.dt.float8e4
    PM = mybir.MatmulPerfMode
    xT = nc.declare_dram_parameter("xT", [BPC, P, DC, N], fmm, isOutput=False)
    if fp8:
        x8d = nc.declare_dram_parameter("x8", [BPC, P, DC, N], f8,
                                        isOutput=False)
        wq8d = nc.declare_dram_parameter("wq8", [P, DC, D], f8, isOutput=False)
        wk8d = nc.declare_dram_parameter("wk8", [P, DC, D], f8, isOutput=False)
    else:
        wqT = nc.declare_dram_parameter("wqT", [P, DC, D], fmm, isOutput=False)
        wkT = nc.declare_dram_parameter("wkT", [P, DC, D], fmm, isOutput=False)
    wvT = nc.declare_dram_parameter("wvT", [P, DC, D], fmm, isOutput=False)
    pbT = nc.declare_dram_parameter("pbT", [N, N], f32, isOutput=False)
    if with_bias:
        bias = nc.declare_dram_parameter("bias", [3, D], fmm, isOutput=False)
    out = nc.declare_dram_parameter("out", [BPC, N, D], f32, isOutput=True)

    with tile.TileContext(nc) as tc, ExitStack() as ctx:
        wpool = ctx.enter_context(tc.tile_pool(name="w", bufs=1))
        ebpool = ctx.enter_context(tc.tile_pool(name="eb", bufs=1))
        stg = ctx.enter_context(tc.tile_pool(name="stg", bufs=2))
        xpool = ctx.enter_context(tc.tile_pool(name="x", bufs=2))
        if fp8:
            x8pool = ctx.enter_context(tc.tile_pool(name="x8", bufs=2))
        ekpool = ctx.enter_context(tc.tile_pool(name="ek", bufs=2))
        ekvpool = ctx.enter_context(tc.tile_pool(name="ekv", bufs=2))
        spool = ctx.enter_context(tc.tile_pool(name="small", bufs=2))
        opool = ctx.enter_context(tc.tile_pool(name="out", bufs=3))
        ps1 = ctx.enter_context(
            tc.tile_pool(name="ps1", bufs=4, space=bass.MemorySpace.PSUM))
        ps2 = ctx.enter_context(
            tc.tile_pool(name="ps2", bufs=4, space=bass.MemorySpace.PSUM))

        # ---- replicated constants -------------------------------------
        # weights stored [p, chunk, e]: partition = d within chunk.
        # Chunked DMAs so the first matmul only waits on ~512KB, not 7MB.
        # issue the startup DMAs from different engines so the ~600ns
        # issue instructions don't serialize on one queue
        wv_t = wpool.tile([P, DC, D], fmm, tag="wv")
        if fp8:
            wq_t = wpool.tile([P, DC, D], f8, tag="wq")
            wk_t = wpool.tile([P, DC, D], f8, tag="wk")
            nc.sync.dma_start(wk_t[:], wk8d.ap())
        else:
            wq_t = wpool.tile([P, DC, D], fmm, tag="wq")
            wk_t = wpool.tile([P, DC, D], fmm, tag="wk")
            nc.sync.dma_start(wk_t[:], wkT.ap())

        if with_bias:
            b_t = wpool.tile([1, 3, D], fmm, tag="bias")
            nc.sync.dma_start(b_t[:], bias.ap().rearrange("t e -> 1 t e"))
            ones_t = wpool.tile([1, P], fmm, tag="ones")
            nc.gpsimd.memset(ones_t[:], 1.0)

        eb_t = ebpool.tile([P, NT, N], fmm, tag="ebt")

        # ---- per-batch pipeline ---------------------------------------
        for b in range(BPC):
            xt = xpool.tile([P, DC, N], fmm, tag="xt")
            if fp8:
                x8t = x8pool.tile([P, DC, N], f8, tag="x8t")
            if b == 0:
                # first batch: spread the startup set (k/q operands first,
                # then v operands) over all three DMA issue queues in
                # consumption order; per-queue bandwidth is only ~145GB/s
                xv = xT.ap()[b]
                if fp8:
                    nc.scalar.dma_start(x8t[:], x8d.ap()[b])
                    nc.gpsimd.dma_start(xt[:, :2, :], xv[:, :2, :])
                    nc.sync.dma_start(xt[:, 2:, :], xv[:, 2:, :])
                    nc.gpsimd.dma_start(wv_t[:], wvT.ap())
                else:
                    nc.scalar.dma_start(xt[:, 0, :], xv[:, 0, :])
                    nc.gpsimd.dma_start(xt[:, 1, :], xv[:, 1, :])
                    nc.scalar.dma_start(xt[:, 2, :], xv[:, 2, :])
                    nc.sync.dma_start(xt[:, 3, :], xv[:, 3, :])
                    nc.gpsimd.dma_start(wv_t[:], wvT.ap())
            else:
                nc.sync.dma_start(xt[:], xT.ap()[b])
                if fp8:
                    nc.scalar.dma_start(x8t[:], x8d.ap()[b])

            ek = ekpool.tile([P, NT, D], fmm, tag="ek")
            ekv = ekvpool.tile([P, NT, D], fmm, tag="ekv")

            # stage 1: k, v projections; ek = exp(k); ekv = ek * v
            for t in range(NT):
                kps = ps1.tile([P, D], f32, tag="ps1")
                if fp8:
                    for c in range(DC // 2):
                        nc.tensor.matmul(
                            kps[:], x8t[:, 2 * c:2 * c + 2, t * P:(t + 1) * P],
                            wk_t[:, 2 * c:2 * c + 2, :],
                            start=(c == 0), stop=(c == DC // 2 - 1),
                            perf_mode=PM.DoubleRow)
                else:
                    for dc in range(DC):
                        nc.tensor.matmul(
                            kps[:], mm_ap(xt[:, dc, t * P:(t + 1) * P]),
                            mm_ap(wk_t[:, dc, :]),
                            start=(dc == 0),
                            stop=(dc == DC - 1 and not with_bias))
                if with_bias:
                    nc.tensor.matmul(
                        kps[:], mm_ap(ones_t[0:1, :]), mm_ap(b_t[0:1, 1, :]),
                        start=False, stop=True)
                vps = ps1.tile([P, D], f32, tag="ps1")
                for dc in range(DC):
                    nc.tensor.matmul(
                        vps[:], mm_ap(xt[:, dc, t * P:(t + 1) * P]),
                        mm_ap(wv_t[:, dc, :]),
                        start=(dc == 0), stop=(dc == DC - 1 and not with_bias))
                if with_bias:
                    nc.tensor.matmul(
                        vps[:], mm_ap(ones_t[0:1, :]), mm_ap(b_t[0:1, 2, :]),
                        start=False, stop=True)
                nc.scalar.activation(ek[:, t, :], kps[:], AF.Exp,
                                     scale=(1.0 / FP8_SCALE) if fp8 else 1.0)
                nc.vector.tensor_mul(ekv[:, t, :], vps[:], ek[:, t, :])

                if b == 0 and t == 1:
                    # deferred constants: needed from stage 2 onwards;
                    # emitting them here keeps the startup DMA critical path
                    # minimal while still landing before stage 2 needs them.
                    # pos_bias rides the gpsimd SWDGE queue.
                    nc.sync.dma_start(wq_t[:],
                                      wq8d.ap() if fp8 else wqT.ap())
                    for jc in range(NT):
                        pb_stage = stg.tile([P, N], f32, tag="pbstg")
                        nc.gpsimd.dma_start(
                            pb_stage[:], pbT.ap()[jc * P:(jc + 1) * P, :])
                        nc.scalar.activation(
                            eb_t[:, jc, :], pb_stage[:], AF.Exp)

            # stage 2: q first (so sigmoid overlaps den/num matmuls),
            # then den = eb@ek and num = eb@ekv; combine and store
            for t in range(NT):
                qps = ps1.tile([P, D], f32, tag="ps1")
                if fp8:
                    for c in range(DC // 2):
                        nc.tensor.matmul(
                            qps[:], x8t[:, 2 * c:2 * c + 2, t * P:(t + 1) * P],
                            wq_t[:, 2 * c:2 * c + 2, :],
                            start=(c == 0), stop=(c == DC // 2 - 1),
                            perf_mode=PM.DoubleRow)
                else:
                    for dc in range(DC):
                        nc.tensor.matmul(
                            qps[:], mm_ap(xt[:, dc, t * P:(t + 1) * P]),
                            mm_ap(wq_t[:, dc, :]),
                            start=(dc == 0),
                            stop=(dc == DC - 1 and not with_bias))
                if with_bias:
                    nc.tensor.matmul(
                        qps[:], mm_ap(ones_t[0:1, :]), mm_ap(b_t[0:1, 0, :]),
                        start=False, stop=True)
                sig = spool.tile([P, D], f32, tag="sig")
                nc.scalar.activation(sig[:], qps[:], AF.Sigmoid,
                                     scale=(1.0 / FP8_SCALE) if fp8 else 1.0)
                # den/num interleaved per j-chunk: adjacent matmuls share the
                # same stationary ebT tile, so walrus ldw-opt can drop every
                # second LDWEIGHTS
                dps = ps2.tile([P, D], f32, tag="ps2")
                nps = ps2.tile([P, D], f32, tag="ps2")
                for jc in range(NT):
                    nc.tensor.matmul(
                        dps[:], mm_ap(eb_t[:, jc, t * P:(t + 1) * P]),
                        mm_ap(ek[:, jc, :]),
                        start=(jc == 0), stop=(jc == NT - 1))
                    nc.tensor.matmul(
                        nps[:], mm_ap(eb_t[:, jc, t * P:(t + 1) * P]),
                        mm_ap(ekv[:, jc, :]),
                        start=(jc == 0), stop=(jc == NT - 1))
                rec = spool.tile([P, D], f32, tag="rec")
                nc.vector.reciprocal_approx_fast(rec[:], dps[:])
                ot = opool.tile([P, D], f32, tag="ot")
                nc.vector.tensor_mul(ot[:], nps[:], rec[:])
                nc.vector.tensor_mul(ot[:], ot[:], sig[:])
                nc.sync.dma_start(out.ap()[b, t * P:(t + 1) * P, :], ot[:])

    nc.compile()
    return nc


_LDW_OPT = False


def _patch_ldw_opt():
    """Flip walrus's --enable-ldw-opt so duplicate LDWEIGHTS of the same
    stationary tile (den/num pairs in stage 2) collapse to one load."""
    import concourse.bass_utils as bu
    if getattr(bu, "_aft_ldw_patched", False):
        return
    orig = bu.run_command

    def patched(cmd, *a, **kw):
        if isinstance(cmd, list):
            cmd = [c.replace("--enable-ldw-opt=false", "--enable-ldw-opt=true")
                   if isinstance(c, str) else c for c in cmd]
        return orig(cmd, *a, **kw)

    bu.run_command = patched
    bu._aft_ldw_patched = True


def _run(inputs, trace=False, **spmd_kwargs):
    from concourse.bass_utils import run_bass_kernel_spmd
    if _LDW_OPT:
        _patch_ldw_opt()

    x = np.ascontiguousarray(np.asarray(inputs["x"], dtype=np.float32))
    Wq = np.asarray(inputs["Wq"], dtype=np.float32)
    Wk = np.asarray(inputs["Wk"], dtype=np.float32)
    Wv = np.asarray(inputs["Wv"], dtype=np.float32)
    bq = np.asarray(inputs["bq"], dtype=np.float32)
    bk = np.asarray(inputs["bk"], dtype=np.float32)
    bv = np.asarray(inputs["bv"], dtype=np.float32)
    pb = np.asarray(inputs["pos_bias"], dtype=np.float32)

    if MM_MODE == "bf16":
        import ml_dtypes
        _mt = ml_dtypes.bfloat16
    else:
        _mt = np.float32

    def _perm(wT):
        # [D, cols] -> [P, DC, cols] with dev[p, c, :] = wT[c*P + p, :]
        cols = wT.shape[1]
        return np.ascontiguousarray(
            wT.reshape(DC, P, cols).transpose(1, 0, 2)).astype(_mt)

    # x[b].T pre-permuted: xT[b, p, c, n] = x[b].T[c*P + p, n]
    xT = np.ascontiguousarray(
        x.transpose(0, 2, 1).reshape(BS, DC, P, N).transpose(0, 2, 1, 3)
    ).astype(_mt)                                                # [BS, P, DC, N]
    wqT = _perm(Wq.T)                                            # [P, DC, D]
    wkT = _perm(Wk.T)
    wvT = _perm(Wv.T)
    pbT = np.ascontiguousarray(pb.T)                             # [j, i]

    with_bias = bool(np.any(bq) or np.any(bk) or np.any(bv))
    fp8 = FP8_PROJ and not with_bias
    if fp8:
        import ml_dtypes
        _f8 = ml_dtypes.float8_e4m3
        x8 = xT.astype(np.float32).astype(_f8)
        wq8 = (wqT.astype(np.float32) * FP8_SCALE).astype(_f8)
        wk8 = (wkT.astype(np.float32) * FP8_SCALE).astype(_f8)
    key = ("nc", with_bias, MM_MODE, fp8)
    if key not in _CACHE:
        _CACHE[key] = _build(with_bias, fp8)
    nc = _CACHE[key]

    in_maps = []
    for c in range(NCORES):
        m = {
            "xT": xT[c * BPC:(c + 1) * BPC],
            "wvT": wvT,
            "pbT": pbT,
        }
        if fp8:
            m["x8"] = x8[c * BPC:(c + 1) * BPC]
            m["wq8"] = wq8
            m["wk8"] = wk8
        else:
            m["wqT"] = wqT
            m["wkT"] = wkT
        if with_bias:
            m["bias"] = np.ascontiguousarray(np.stack([bq, bk, bv])).astype(_mt)
        in_maps.append(m)

    res = run_bass_kernel_spmd(nc, in_maps, core_ids=list(range(NCORES)),
                               trace=trace, **spmd_kwargs)
    out = np.concatenate([r["out"] for r in res.results], axis=0)
    return out.astype(np.float32, copy=False), res


def kernel(**inputs) -> np.ndarray:
    out, _ = _run(inputs, trace=False)
    return out
